# revision 39
# baseline (speedup 1.0000x reference)
"""CARC attention processor kernel for 8 Trainium2 NeuronCores.

Sharding: data-parallel over the fused B*H axis. 80 heads / 8 cores =
10 heads per core; each core owns one batch (bi = core//2) and one
10-head group (g = core%2). Projection weights are column/row-sliced
per head group; the KV bank is sliced per core. Each core emits a
partial output projection over its 640 channels; the host sums the two
partials per batch and adds the bias.

Device algorithm per core (fp16 matmuls, fp32 PSUM):
  - qT/kT projections in transposed layout [64*heads, L] (Dh on
    partitions) so scores contract over Dh directly.
  - the attention core runs per (pair, q-half): queries are processed
    in two 512-wide passes so a kc's scores for BOTH heads fit one
    [128, 1024] PSUM tile (par0 cols 0:512, par1 512:1024, different
    banks).  The two score matmuls (K=64, tile_position (0,0)/(64,0))
    are emitted adjacently and become simultaneously ready, so the PE
    row-tiles run them concurrently.
  - one ACT exp per kc over the whole [128, 1024] tile (scale 1/8
    fused; no max subtraction: |scores| < ~6).
  - ctx matmuls accumulate [128 = 64 v | 64 ones, 512 q] per head;
    the ones columns produce the softmax denominator for free.
  - normalization = DVE reciprocal of the denominator rows x ctx rows,
    fused into the PSUM->SBUF evacuation (off critical path except the
    final pass).
  - output projection contracts head pairs (K=128) of ctxT against
    row-slices of Wo; qt tiles 0-3 (first q-half) are absorbed into
    the last pair's second pass.

PSUM budget (16KB/partition): scores 2x[128,1024]f32 (8K) + ctx
2x[128,512]f32 (4K) + projection slots 2x[128,512]f32 (4K).  The
projection slots let all q/k/v projections for later pairs (and the
early out-projection tiles) absorb into the attention loop's PE slack,
which is otherwise idle while ACT runs the exps.
"""
from contextlib import ExitStack

import numpy as np

import concourse.bass as bass
import concourse.tile as tile
from concourse import bacc, mybir
from concourse import bass_utils

F32 = mybir.dt.float32
F16 = mybir.dt.float16
ActF = mybir.ActivationFunctionType

B, L, C, H, Dh = 4, 1024, 1280, 20, 64
NCORES = 8
HPC = 10               # heads per core
NP = HPC // 2          # head pairs per core
ALPHA = 0.8 * 0.6
LB = 256               # bank keys per head after 2x2 pooling
KEYS = L + LB          # 1280
KCH = KEYS // 128      # 10 key chunks
CC = C // 128          # 10 contraction chunks
LT = L // 128          # 8 key tiles from the projected keys
QW = 512               # query-pass width


def _build():
    nc = bacc.Bacc("TRN2", target_bir_lowering=False, debug=False,
                   num_devices=NCORES)
    hsT_d = nc.dram_tensor("hsT", [C, L], F16, kind="ExternalInput")
    # wq/wk pre-arranged on host as [NP][128 part][CC][128 cols]
    wq_d = nc.dram_tensor("wq", [NP, 128, CC, 128], F16, kind="ExternalInput")
    wk_d = nc.dram_tensor("wk", [NP, 128, CC, 128], F16, kind="ExternalInput")
    # wv pre-arranged as [2 halves][128 part][CC][320 cols]
    wv_d = nc.dram_tensor("wv", [2, 128, CC, 320], F16, kind="ExternalInput")
    wo_d = nc.dram_tensor("wo", [HPC * Dh, C], F16, kind="ExternalInput")
    kbT_d = nc.dram_tensor("kbT", [HPC * Dh, LB], F16, kind="ExternalInput")
    vb_d = nc.dram_tensor("vb", [LB, HPC * Dh], F16, kind="ExternalInput")
    out_d = nc.dram_tensor("out", [L, C], F16, kind="ExternalOutput")

    with tile.TileContext(nc) as tc, ExitStack() as es:
        big = es.enter_context(tc.tile_pool(name="big", bufs=1))
        wst = es.enter_context(tc.tile_pool(name="wst", bufs=2))
        qkt = es.enter_context(tc.tile_pool(name="qkt", bufs=2))
        expp = es.enter_context(tc.tile_pool(name="expp", bufs=4))
        nrm = es.enter_context(tc.tile_pool(name="nrm", bufs=2))
        outp = es.enter_context(tc.tile_pool(name="outp", bufs=3))
        scp = es.enter_context(tc.tile_pool(name="scp", bufs=2, space="PSUM"))
        ctp = es.enter_context(tc.tile_pool(name="ctp", bufs=1, space="PSUM"))
        prp = es.enter_context(tc.tile_pool(name="prp", bufs=2, space="PSUM"))

        ctxT_sb = big.tile([128, NP, L], F16)
        v_sb = big.tile([128, KCH, HPC * 128], F16)
        v_heads = v_sb[:].rearrange("p c (h x) -> p c h x", x=128)
        hsT_sb = big.tile([128, CC, L], F16)

        # ---- phase 0: input DMAs, spread across engine queues.  Weights
        # for pair 0 first (whole-tile deps), then hsT chunks round-robin
        # so the cc-major phase-1 chains unblock per-chunk.  Bank/wv1
        # DMAs are deferred into the loop (HBM bandwidth is the phase-1
        # critical path).
        wq_tiles, wk_tiles = {}, {}
        wq_tiles[0] = wst.tile([128, CC, 128], F16, tag="wq", name="wq0")
        wk_tiles[0] = wst.tile([128, CC, 128], F16, tag="wk", name="wk0")
        wv_tiles = [wst.tile([128, CC, 320], F16, tag=f"wv{g}", bufs=1,
                             name=f"wv{g}")
                    for g in range(2)]
        # hsT chunks all on ONE queue so they complete in order (the DMA
        # hardware round-robins concurrent rings, which would delay chunk
        # 0 to the end of the whole transfer).  wv0 first on gpsimd (the
        # phase-1 v chains start on it), wq/wk interleaved behind the
        # first hsT chunk on sync.
        nc.gpsimd.dma_start(wv_tiles[0][:], wv_d.ap()[0])
        for cc in range(CC):
            nc.sync.dma_start(hsT_sb[:, cc, :],
                              hsT_d.ap()[cc * 128:(cc + 1) * 128, :])
            if cc == 1:
                nc.scalar.dma_start(wq_tiles[0][:], wq_d.ap()[0])
                nc.scalar.dma_start(wk_tiles[0][:], wk_d.ap()[0])
        for kc in range(KCH):
            nc.gpsimd.memset(v_heads[:, kc, :, Dh:128], 1.0)

        def emit_late_dmas():
            nc.gpsimd.dma_start(wv_tiles[1][:], wv_d.ap()[1])
            for j in range(LB // 128):
                nc.gpsimd.dma_start(
                    v_heads[:, LT + j, :, 0:Dh],
                    vb_d.ap()[j * 128:(j + 1) * 128, :]
                    .rearrange("p (h d) -> p h d", d=Dh))

        qts, kts = {}, {}

        def emit_qk_proj(m, which, w_dma=True):
            """q or k projection for pair m: 2 chains of CC matmuls into
            [128, 512] PSUM slots, evacuated to the transposed SBUF tile."""
            if which == "q":
                dst = qts[m] = qkt.tile([128, L], F16, tag="qT", name=f"qT{m}")
                w_d, wtag, wt = wq_d, "wq", wq_tiles
            else:
                dst = kts[m] = qkt.tile([128, KEYS], F16, tag="kT",
                                        name=f"kT{m}")
                w_d, wtag, wt = wk_d, "wk", wk_tiles
            if w_dma:
                wt[m] = wst.tile([128, CC, 128], F16, tag=wtag,
                                 name=f"{wtag}{m}")
                (nc.gpsimd if which == "q" else nc.sync).dma_start(
                    wt[m][:], w_d.ap()[m])
            w_sb = wt[m]
            for qh in range(2):
                pp = prp.tile([128, QW], F32, tag="pj", name=f"p{wtag}{m}_{qh}")
                for cc in range(CC):
                    nc.tensor.matmul(
                        pp[:],
                        w_sb[:, cc, :],
                        hsT_sb[:, cc, qh * 512:(qh + 1) * 512],
                        start=(cc == 0), stop=(cc == CC - 1))
                nc.vector.tensor_copy(dst[:, qh * 512:(qh + 1) * 512], pp[:])
            if which == "k":
                nc.gpsimd.dma_start(dst[:, L:KEYS],
                                    kbT_d.ap()[m * 128:(m + 1) * 128, :])

        def vproj_mm(pv, c0, g, lt, cc):
            nc.tensor.matmul(
                pv[:, c0:c0 + 320],
                hsT_sb[:, cc, lt * 128:(lt + 1) * 128],
                wv_tiles[g][:, cc, :],
                start=(cc == 0), stop=(cc == CC - 1))

        def vproj_evac(pv, c0, g, lt):
            nc.vector.tensor_copy(
                v_heads[:, lt, g * 5:(g + 1) * 5, 0:Dh],
                pv[:, c0:c0 + 320].rearrange("p (h d) -> p h d", d=Dh))

        def emit_vproj_lt(g, lt):
            """v projection for key tile lt, head half g (5 heads)."""
            pv = prp.tile([128, QW], F32, tag="pj", name=f"pv{g}_{lt}")
            for cc in range(CC):
                vproj_mm(pv, 0, g, lt, cc)
            vproj_evac(pv, 0, g, lt)

        wo_tiles = {}

        def emit_wo_dma(p):
            wo_sb = wst.tile([128, C], F16, tag=f"wo{p}", bufs=1)
            (nc.sync if p % 2 == 0 else nc.gpsimd).dma_start(
                wo_sb[:], wo_d.ap()[p * 128:(p + 1) * 128, :])
            wo_tiles[p] = wo_sb

        out_chunks = [(0, 512), (512, 512), (1024, 256)]
        out_dma_q = [nc.sync, nc.gpsimd, nc.scalar]
        out_n = [0]

        def emit_outproj(qt, n0, nsz, tail=False):
            po = prp.tile([128, QW], F32, tag="pj", name=f"po{qt}_{n0}")
            # in the tail, pair 2 (processed last) goes last in the chain
            # so the other four matmuls can run before its norm lands
            order = (0, 1, 3, 4, 2) if tail else tuple(range(NP))
            for j, p in enumerate(order):
                nc.tensor.matmul(
                    po[:, 0:nsz],
                    ctxT_sb[:, p, qt * 128:(qt + 1) * 128],
                    wo_tiles[p][:, n0:n0 + nsz],
                    start=(j == 0), stop=(j == NP - 1))
            ob = outp.tile([128, 512], F16, tag="ob", name=f"ob{qt}_{n0}")
            # tail evacuations all go to ACT (idle there) so DVE norm
            # work never blocks the po slot recycling
            if tail:
                nc.scalar.activation(ob[:, 0:nsz], po[:, 0:nsz], ActF.Copy)
            else:
                nc.vector.tensor_copy(ob[:, 0:nsz], po[:, 0:nsz])
            out_dma_q[out_n[0] % 3].dma_start(
                out_d.ap()[qt * 128:(qt + 1) * 128, n0:n0 + nsz],
                ob[:, 0:nsz])
            out_n[0] += 1

        # Pair processing order: pair 2 (which needs both wv halves) goes
        # last so its passes absorb the first-half output projection.
        PO = [0, 1, 3, 4, 2]

        # absorbed-work dispatch: (pass_index, kc) -> list of closures
        absorbed = {}

        def absorb(pi, kc, fn):
            absorbed.setdefault((pi, kc), []).append(fn)

        # p1 (0,0): v half-0 tiles lt 2..7, just in time (ctx(lt) runs at
        # slot lt+2); late DMAs for wv1/bank-v kick off here too.
        absorb(0, 0, emit_late_dmas)
        for lt in range(2, LT):
            absorb(0, lt - 2, (lambda lt=lt: emit_vproj_lt(0, lt)))
        # p2 (0,1): q/k projections for pair 1
        absorb(1, 2, (lambda: emit_qk_proj(1, "q")))
        absorb(1, 6, (lambda: emit_qk_proj(1, "k")))
        # p3/p4 (pair 1): v half-1 lt 0..3 + q/k for pair 3
        absorb(2, 2, (lambda: emit_vproj_lt(1, 0)))
        absorb(2, 6, (lambda: emit_vproj_lt(1, 1)))
        absorb(2, 8, (lambda: emit_qk_proj(3, "q")))
        absorb(3, 2, (lambda: emit_vproj_lt(1, 2)))
        absorb(3, 5, (lambda: emit_qk_proj(3, "k")))
        absorb(3, 8, (lambda: emit_vproj_lt(1, 3)))
        # p5 (3,0): v half-1 lt 4..7 just in time, then q for pair 4
        for lt in range(4, LT):
            absorb(4, lt - 2, (lambda lt=lt: emit_vproj_lt(1, lt)))
        absorb(4, 8, (lambda: emit_qk_proj(4, "q")))
        # p6 (3,1): k for pair 4
        absorb(5, 5, (lambda: emit_qk_proj(4, "k")))
        # p7/p8 (pair 4): q/k for pair 2; wo loads
        absorb(6, 3, (lambda: emit_qk_proj(2, "q")))
        absorb(7, 3, (lambda: emit_qk_proj(2, "k")))
        for p in range(NP):
            absorb(7, 5 + (p % 4), (lambda p=p: emit_wo_dma(p)))
        # p9 (2,0): the last two first-half outproj chunks start early —
        # their pair-0/1/3/4 partial chains fill the otherwise-idle
        # kc8/kc9 boundary slots (all four pairs are normalized by
        # then); the pair-2 matmul completes each chain in (2,1).
        tail_po = {}

        def emit_op_prefix(qt, n0, nsz):
            po = prp.tile([128, QW], F32, tag="pj", name=f"pop{qt}_{n0}")
            for j, p in enumerate((0, 1, 3, 4)):
                nc.tensor.matmul(
                    po[:, 0:nsz],
                    ctxT_sb[:, p, qt * 128:(qt + 1) * 128],
                    wo_tiles[p][:, n0:n0 + nsz],
                    start=(j == 0), stop=False)
            tail_po[(qt, n0)] = po

        def emit_op_final(qt, n0, nsz):
            po = tail_po[(qt, n0)]
            nc.tensor.matmul(
                po[:, 0:nsz],
                ctxT_sb[:, 2, qt * 128:(qt + 1) * 128],
                wo_tiles[2][:, n0:n0 + nsz],
                start=False, stop=True)
            ob = outp.tile([128, 512], F16, tag="ob", name=f"obf{qt}_{n0}")
            nc.vector.tensor_copy(ob[:, 0:nsz], po[:, 0:nsz])
            out_dma_q[out_n[0] % 3].dma_start(
                out_d.ap()[qt * 128:(qt + 1) * 128, n0:n0 + nsz],
                ob[:, 0:nsz])
            out_n[0] += 1

        oc = [(qt, n0, nsz) for qt in range(4) for (n0, nsz) in out_chunks]
        for j, (qt, n0, nsz) in enumerate(oc[:10]):
            absorb(9, min(j + 1, KCH - 1),
                   (lambda qt=qt, n0=n0, nsz=nsz: emit_outproj(qt, n0, nsz)))
        for j, (qt, n0, nsz) in enumerate(oc[10:]):
            absorb(8, KCH - 2 + j,
                   (lambda qt=qt, n0=n0, nsz=nsz: emit_op_prefix(qt, n0, nsz)))
            absorb(9, 0,
                   (lambda qt=qt, n0=n0, nsz=nsz: emit_op_final(qt, n0, nsz)))

        # ---- phase 1: pair-0 q/k/v projections, cc-major so each hsT
        # chunk arrival unblocks 6 matmuls.  The q chains borrow the ctx
        # PSUM slots and the v chains borrow a scores slot (the loop
        # reuses them afterwards through the normal pool rings).
        qts[0] = qkt.tile([128, L], F16, tag="qT", name="qT0")
        kts[0] = qkt.tile([128, KEYS], F16, tag="kT", name="kT0")
        q_ch = [ctp.tile([128, QW], F32, tag=f"c{qh}", name=f"ph1_q{qh}")
                for qh in range(2)]
        k_ch = [prp.tile([128, QW], F32, tag="pj", name=f"ph1_k{qh}")
                for qh in range(2)]
        v_ch = scp.tile([128, 1024], F32, tag="T", name="ph1_v")
        for cc in range(CC):
            for lt in range(2):
                vproj_mm(v_ch, lt * 512, 0, lt, cc)
            for qh in range(2):
                nc.tensor.matmul(
                    q_ch[qh][:], wq_tiles[0][:, cc, :],
                    hsT_sb[:, cc, qh * 512:(qh + 1) * 512],
                    start=(cc == 0), stop=(cc == CC - 1))
                nc.tensor.matmul(
                    k_ch[qh][:], wk_tiles[0][:, cc, :],
                    hsT_sb[:, cc, qh * 512:(qh + 1) * 512],
                    start=(cc == 0), stop=(cc == CC - 1))
        # first-half evacs first: scores kc0 needs only qT/kT cols 0:512
        nc.vector.tensor_copy(qts[0][:, 0:512], q_ch[0][:])
        nc.vector.tensor_copy(kts[0][:, 0:512], k_ch[0][:])
        nc.vector.tensor_copy(qts[0][:, 512:1024], q_ch[1][:])
        nc.vector.tensor_copy(kts[0][:, 512:1024], k_ch[1][:])
        for lt in range(2):
            vproj_evac(v_ch, lt * 512, 0, lt)
        nc.gpsimd.dma_start(kts[0][:, L:KEYS], kbT_d.ap()[0:128, :])

        # ---- phase 2: attention loop ----
        def emit_ctx(m, ps, kc, cps, es_tile):
            for par in range(2):
                nc.tensor.matmul(
                    cps[par][:],
                    v_heads[:, kc, 2 * m + par, :],
                    es_tile[kc][:, par * QW:(par + 1) * QW],
                    start=(kc == 0), stop=(kc == KCH - 1))

        for pi in range(2 * NP):
            m, ps = PO[pi // 2], pi % 2
            if True:
                q0 = ps * QW
                cps = [ctp.tile([128, QW], F32, tag=f"c{par}",
                                name=f"c{m}_{ps}_{par}")
                       for par in range(2)]
                e_tiles = {}
                # kc in steps of 2: batching the two kc's score matmuls
                # (64-row tiling mode) and the two ctx matmul pairs (full
                # 128 mode) halves the PE tiling-mode switch drains.
                for kc0 in range(0, KCH, 2):
                    for kc in (kc0, kc0 + 1):
                        ts = scp.tile([128, 1024], F32, tag="T",
                                      name=f"T{m}_{ps}_{kc}")
                        for par in range(2):
                            p0 = 64 * par
                            nc.tensor.matmul(
                                ts[:, par * QW:(par + 1) * QW],
                                kts[m][p0:p0 + 64, kc * 128:(kc + 1) * 128],
                                qts[m][p0:p0 + 64, q0:q0 + QW],
                                start=True, stop=True, tile_position=(p0, 0))
                        e = expp.tile([128, 1024], F16, tag="e",
                                      name=f"e{m}_{ps}_{kc}")
                        nc.scalar.activation(e[:], ts[:], ActF.Exp,
                                             scale=0.125)
                        e_tiles[kc] = e
                    for kc in (kc0, kc0 + 1):
                        if kc >= 2:
                            emit_ctx(m, ps, kc - 2, cps, e_tiles)
                        for fn in absorbed.get((pi, kc), ()):
                            fn()
                emit_ctx(m, ps, KCH - 2, cps, e_tiles)
                emit_ctx(m, ps, KCH - 1, cps, e_tiles)
                # normalization + evacuation (chunked on the last pass so
                # the first outproj qt tiles unblock early).  The approx
                # reciprocal runs over all 128 partitions — it miscomputes
                # under partition-offset APs, so feed it base-0 input; the
                # ctx-row half of the result is garbage and never read.
                rcs = [nrm.tile([128, QW], F32, tag=f"rc{par}",
                                name=f"rc{m}_{ps}_{par}") for par in range(2)]
                chunks = ((0, 128), (128, 128), (256, 128), (384, 128)) \
                    if pi == 2 * NP - 1 else ((0, QW),)
                for c0, csz in chunks:
                    for par in range(2):
                        sl = slice(64 * par, 64 * par + 64)
                        cl = slice(c0, c0 + csz)
                        nc.vector.reciprocal_approx_fast(
                            rcs[par][:, cl], cps[par][:, cl])
                        nc.vector.tensor_mul(
                            ctxT_sb[sl, m, q0 + c0:q0 + c0 + csz],
                            cps[par][0:64, cl],
                            rcs[par][64:128, cl])

        # ---- phase 3: remaining output projection (qt 4-7) ----
        for qt in range(4, LT):
            for (n0, nsz) in out_chunks:
                emit_outproj(qt, n0, nsz, tail=True)
    nc.compile()
    return nc


_NC = None


def _get_nc():
    global _NC
    if _NC is None:
        _NC = _build()
    return _NC


def _prep_in_maps(hidden_states, Wq, Wk, Wv, Wo, K_bg, V_bg):
    hs = np.asarray(hidden_states, np.float32)
    Wq, Wk, Wv, Wo = (np.asarray(w, np.float32) for w in (Wq, Wk, Wv, Wo))
    K_bg = np.asarray(K_bg, np.float32)
    V_bg = np.asarray(V_bg, np.float32)

    hsT = [np.ascontiguousarray(hs[bi].T).astype(np.float16)
           for bi in range(B)]

    def lay_qk(w, g):  # [1280, 640] slice -> [NP, 128, CC, 128]
        sl = w[:, g * 640:(g + 1) * 640]           # [C, 640]
        a = sl.reshape(CC, 128, NP, 128)           # (cc, p, m, n)
        return np.ascontiguousarray(a.transpose(2, 1, 0, 3)).astype(np.float16)

    def lay_wv(w, g):  # [1280, 640] slice -> [2, 128, CC, 320]
        sl = w[:, g * 640:(g + 1) * 640]
        a = sl.reshape(CC, 128, 2, 320)            # (cc, p, gg, n)
        return np.ascontiguousarray(a.transpose(2, 1, 0, 3)).astype(np.float16)

    wq_s = [lay_qk(Wq, g) for g in range(2)]
    wk_s = [lay_qk(Wk, g) for g in range(2)]
    wv_s = [lay_wv(Wv, g) for g in range(2)]
    wo_s = [Wo[g * 640:(g + 1) * 640, :].astype(np.float16) for g in range(2)]

    def pool_bank(x):  # [10, 1024, 64] -> [10, 256, 64], fp16 round + alpha
        x = x.astype(np.float16).astype(np.float32)
        x = x.reshape(HPC, 16, 2, 16, 2, Dh).mean(axis=(2, 4))
        return (ALPHA * x).reshape(HPC, LB, Dh)

    kb_s, vb_s = [], []
    for base in (0, 10, 20, 30):
        kb = pool_bank(K_bg[base:base + HPC])
        vb = pool_bank(V_bg[base:base + HPC])
        kb_s.append(kb.transpose(0, 2, 1).reshape(HPC * Dh, LB).astype(np.float16))
        vb_s.append(vb.transpose(1, 0, 2).reshape(LB, HPC * Dh).astype(np.float16))

    in_maps = []
    for c in range(NCORES):
        bi, g = c // 2, c % 2
        bank = (20 * bi + 10 * g) % 40 // 10
        in_maps.append({
            "hsT": hsT[bi], "wq": wq_s[g], "wk": wk_s[g], "wv": wv_s[g],
            "wo": wo_s[g], "kbT": kb_s[bank], "vb": vb_s[bank],
        })
    return in_maps


def _run(in_maps, **kwargs):
    return bass_utils.run_bass_kernel_spmd(
        _get_nc(), in_maps, core_ids=list(range(NCORES)), **kwargs)


def kernel(hidden_states, Wq, Wk, Wv, Wo, bo, K_bg, V_bg):
    in_maps = _prep_in_maps(hidden_states, Wq, Wk, Wv, Wo, K_bg, V_bg)
    res = _run(in_maps)
    bo = np.asarray(bo, np.float32)
    out = np.empty((B, L, C), np.float32)
    for bi in range(B):
        out[bi] = (res.results[2 * bi]["out"].astype(np.float32)
                   + res.results[2 * bi + 1]["out"].astype(np.float32)
                   + bo[None, :])
    return out
